# revision 1
# baseline (speedup 1.0000x reference)
"""Trainium2 Bass kernel for the DCN (modulated deformable conv) layer.

Self-contained: hardcodes all shapes. Shards data-parallel over (batch b x
row-half h) onto 8 NeuronCores; each core computes a [64, 64, 128] slab of
the [4, 64, 128, 128] output.

Per-core algorithm (all position indices x live on SBUF partitions):
  1. om-conv (3x3, PE, bf16): offsets dy,dx + mask logits from `inter`.
  2. PE-transpose om -> omT[x, y, ch]; build bilinear/sigmoid mask planes
     m[p, sy, sx, y][x] on DVE (positions-in-partitions layout).
  3. PE-transpose feat -> featT5[x, dx_shift, row, c] (5 column shifts via
     DMA partition-shifted copies).
  4. Apply: per (tap, 4-row block): 9 contiguous tensor_tensor products
     (mask broadcast along c via a 0-stride free dim) + an 8-op add chain
     on DVE -- 2448 ops total, all operands contiguous [128, 4, 64] bf16.
  5. PE-transpose valT back to val[(c,p)-partitions, (y,x)] and contract with
     the per-sample dynamic weights W2' = c2_w @ (weight * fvec) on PE.
"""

import numpy as np
from contextlib import ExitStack

import concourse.bass as bass
import concourse.bacc as bacc
import concourse.tile as tile
from concourse import mybir
from concourse.bass_utils import run_bass_kernel_spmd

F32 = mybir.dt.float32
BF16 = mybir.dt.bfloat16
ALU = mybir.AluOpType
ACTF = mybir.ActivationFunctionType

B, CIN, COUT, H, W, K = 4, 64, 64, 128, 128, 3
KK = K * K
NOUT = 64          # out rows per core
NR = 69            # feat rows resident per core:  y + ky-1 + sy in [-3, 66)
NRI = 66           # inter rows resident (conv halo 1)
NCH = 16           # position chunks (of 512) per core
CLAMP = 0.999999

_CACHED = {}


def _build_nc():
    nc = bacc.Bacc("TRN2", target_bir_lowering=False)

    # ---- DRAM I/O (per-core views; same program on all 8 cores) ----
    d_feat = nc.dram_tensor("feat", [CIN, NR, 132], F32, kind="ExternalInput")
    d_inter = nc.dram_tensor("inter", [CIN, NRI, 130], F32, kind="ExternalInput")
    d_w2 = nc.dram_tensor("w2", [128, 5, 64], F32, kind="ExternalInput")
    d_comw = nc.dram_tensor("comw", [CIN, KK, 27], F32, kind="ExternalInput")
    d_comb = nc.dram_tensor("comb", [27, 1], F32, kind="ExternalInput")
    d_c1w = nc.dram_tensor("c1w", [128, 2, COUT], F32, kind="ExternalInput")
    d_fea = nc.dram_tensor("fea", [128, 2], F32, kind="ExternalInput")
    d_bias2 = nc.dram_tensor("bias2", [COUT, 1], F32, kind="ExternalInput")
    d_ident = nc.dram_tensor("ident", [128, 128], F32, kind="ExternalInput")
    d_out = nc.dram_tensor("out", [COUT, NOUT, W], F32, kind="ExternalOutput")

    with ExitStack() as ctx:
        tc = ctx.enter_context(tile.TileContext(nc))

        # ---------------- persistent small pool ----------------
        pers = ctx.enter_context(tc.tile_pool(name="pers", bufs=1))
        identb = pers.tile([128, 128], BF16)
        w2cp = pers.tile([128, 5 * 64], F32)
        w2b = pers.tile([128, 5, 64], BF16)
        comwb = pers.tile([CIN, KK, 27], BF16)
        combc = pers.tile([27, 1], F32)
        c1wb = pers.tile([128, 2, COUT], BF16)
        feab = pers.tile([128, 2], BF16)
        bias2c = pers.tile([COUT, 1], F32)
        fvec = pers.tile([128, 1], F32)
        omT = pers.tile([128, NOUT, 27], BF16)
        masks = pers.tile([128, KK, 3, 3, NOUT], F32)

        identf = pers.tile([128, 128], F32)
        nc.sync.dma_start(out=identf, in_=d_ident[:, :])
        nc.vector.tensor_copy(identb[:, :], identf[:, :])
        nc.sync.dma_start(out=w2cp, in_=d_w2.rearrange("p a b -> p (a b)")[:, :])
        dma_comw = nc.gpsimd.dma_start(out=comwb, in_=d_comw[:, :, :])  # cast f32->bf16
        nc.sync.dma_start(out=combc, in_=d_comb[:, :])
        nc.gpsimd.dma_start(out=c1wb, in_=d_c1w[:, :, :])
        nc.gpsimd.dma_start(out=feab, in_=d_fea[:, :])
        nc.sync.dma_start(out=bias2c, in_=d_bias2[:, :])

        psum_sm = ctx.enter_context(tc.tile_pool(name="psum_sm", bufs=1, space="PSUM"))

        # fvec = c1_w @ fea  -> [64, 1]; replicate to [128, 1]
        ps_fv = psum_sm.tile([COUT, 1], F32)
        for k in range(2):
            nc.tensor.matmul(ps_fv[:, :], c1wb[:, k, :], feab[:, k : k + 1],
                             start=(k == 0), stop=(k == 1))
        nc.scalar.copy(fvec[0:COUT, :], ps_fv[:, :])
        nc.sync.dma_start(out=fvec[COUT:128, :], in_=fvec[0:COUT, :])
        # w2b = (w2cp * fvec[c]) cast to bf16
        nc.vector.tensor_scalar(out=w2b.rearrange("p a b -> p (a b)")[:, :],
                                in0=w2cp[:, :], scalar1=fvec[:, :], scalar2=None,
                                op0=ALU.mult)

        # ---------------- phase 1: om conv + masks ----------------
        with tc.tile_pool(name="omph", bufs=1) as omph:
            interb = omph.tile([CIN, NRI, 130], BF16)
            nc.gpsimd.dma_start(out=interb, in_=d_inter[:, :, :])
            om_sb = omph.tile([27, NOUT, W], BF16)
            with tc.tile_pool(name="ompsum", bufs=2, space="PSUM") as ompsum:
                for n in range(NCH):  # 512-wide position chunks = 4 out rows
                    ps = ompsum.tile([27, 512], F32)
                    y0 = 4 * n
                    for d in range(KK):
                        dy, dx = d // 3, d % 3
                        rhs = interb[:, y0 + dy : y0 + dy + 4, dx : dx + W]
                        nc.tensor.matmul(ps[:, :], comwb[:, d, :], rhs,
                                         start=(d == 0), stop=(d == KK - 1))
                    nc.scalar.activation(om_sb[:, y0 : y0 + 4, :].rearrange("p a b -> p (a b)"),
                                         ps[:, :], ACTF.Identity, bias=combc[:, :])
                # om transpose: [27, 128] row-slices -> omT [128, y, 27]
                for g in range(4):  # 16 rows per psum tile (28-elem slots, 4B-aligned)
                    pst = ompsum.tile([128, 16, 28], BF16)
                    for j in range(16):
                        y = 16 * g + j
                        nc.tensor.transpose(pst[:, j, 0:27],
                                            om_sb[:, y, :], identb[0:27, 0:27])
                    nc.scalar.copy(omT[:, 16 * g : 16 * (g + 1), :], pst[:, :, 0:27])

            # ---- mask build (f32), positions on partitions ----
            mbig = omph.tile([128, 8, KK, NOUT], F32)
            dyT, dxT, sgT, ey, ly, ay, f0, s = [mbig[:, i] for i in range(8)]
            wm_t = omph.tile([128, KK, NOUT], F32, tag="wm")
            wm = wm_t[:, :, :]
            w0_t = omph.tile([128, KK, NOUT], F32, tag="w0")
            w0 = w0_t[:, :, :]
            wp_t = omph.tile([128, KK, NOUT], F32, tag="wp")
            wp = wp_t[:, :, :]
            wys = omph.tile([128, KK, 3, NOUT], F32)
            wxs = omph.tile([128, KK, 3, NOUT], F32)
            # repack dy/dx/sig from omT (ch-minor) into [128, p, y] contiguous
            for dst, lo in [(dyT, 0), (dxT, 9), (sgT, 18)]:
                nc.vector.tensor_copy(dst[:, :, :],
                                      omT[:, :, lo : lo + 9].rearrange("p y c -> p c y"))
            nc.scalar.activation(sgT, sgT, ACTF.Sigmoid)

            for dT, wtile, fold_sig in [(dyT, wys, True), (dxT, wxs, False)]:
                nc.vector.tensor_scalar(out=dT[:, :, :], in0=dT[:, :, :], scalar1=-CLAMP,
                                        scalar2=CLAMP, op0=ALU.max, op1=ALU.min)
                nc.vector.tensor_scalar(out=ey, in0=dT[:, :, :], scalar1=0.0,
                                        scalar2=None, op0=ALU.is_lt)
                nc.vector.tensor_tensor(out=ly, in0=dT[:, :, :], in1=ey,
                                        op=ALU.add)
                nc.vector.tensor_scalar(out=ay, in0=ly, scalar1=-1.0,
                                        scalar2=1.0, op0=ALU.mult, op1=ALU.add)
                nc.vector.tensor_scalar(out=f0, in0=ey, scalar1=-1.0,
                                        scalar2=1.0, op0=ALU.mult, op1=ALU.add)
                nc.vector.tensor_tensor(out=wm, in0=ey, in1=ay,
                                        op=ALU.mult)
                nc.vector.tensor_tensor(out=wp, in0=f0, in1=ly,
                                        op=ALU.mult)
                nc.vector.tensor_tensor(out=s, in0=wm, in1=wp,
                                        op=ALU.add)
                nc.vector.tensor_scalar(out=w0, in0=s, scalar1=-1.0,
                                        scalar2=1.0, op0=ALU.mult, op1=ALU.add)
                for k, wk in enumerate([wm, w0, wp]):
                    if fold_sig:
                        nc.vector.tensor_tensor(out=wtile[:, :, k, :], in0=wk[:, :, :],
                                                in1=sgT, op=ALU.mult)
                    else:
                        nc.vector.tensor_copy(wtile[:, :, k, :], wk[:, :, :])
            # m[p, sy, sx, y] = wys[p, sy, y] * wxs[p, sx, y]
            nc.vector.tensor_tensor(
                out=masks[:, :, :, :, :],
                in0=wys[:, :, :, None, :].broadcast_to([128, KK, 3, 3, NOUT]),
                in1=wxs[:, :, None, :, :].broadcast_to([128, KK, 3, 3, NOUT]),
                op=ALU.mult)

        # ---------------- phase 2: featT5 ----------------
        featT5 = pers.tile([128, 5, NR, CIN], BF16)
        nc.vector.memset(featT5[:, :, :, :], 0.0)
        with tc.tile_pool(name="featph", bufs=1) as featph:
            featb = featph.tile([CIN, NR, 132], BF16)
            nc.gpsimd.dma_start(out=featb, in_=d_feat[:, :, :])
            with tc.tile_pool(name="ftpsum", bufs=4, space="PSUM") as ftpsum:
                for g in range((NR + 7) // 8):  # 8 rows per psum tile
                    rows = range(8 * g, min(8 * g + 8, NR))
                    pst = ftpsum.tile([128, 8 * CIN], BF16)
                    for j, r in enumerate(rows):
                        nc.tensor.transpose(pst[:, CIN * j : CIN * (j + 1)],
                                            featb[:, r, 2 : 2 + 128], identb[0:CIN, 0:CIN])
                    nc.scalar.copy(
                        featT5[:, 2, 8 * g : 8 * g + len(rows), :].rearrange("p a b -> p (a b)"),
                        pst[:, : len(rows) * CIN])
        # shifted copies: featT5[:, 2+d, :, :][x] = featT0[x + d]
        flat = featT5.rearrange("p a b c -> p a (b c)")
        for dlt, di in [(-2, 0), (-1, 1), (1, 3), (2, 4)]:
            if dlt > 0:
                nc.sync.dma_start(out=flat[0 : 128 - dlt, di, :], in_=flat[dlt:128, 2, :])
            else:
                nc.sync.dma_start(out=flat[-dlt : 128, di, :], in_=flat[0 : 128 + dlt, 2, :])

        # ---------------- phase 3: apply + back-transpose + einsum ----------------
        with (
            tc.tile_pool(name="vpool", bufs=2) as vpool,
            tc.tile_pool(name="vblk", bufs=1) as vblk,
            tc.tile_pool(name="och", bufs=2) as och,
            tc.tile_pool(name="vpsum", bufs=2, space="PSUM") as vpsum,
        ):
            BR = 16  # out rows per apply block
            for nb in range(NOUT // BR):
                vt = vpool.tile([128, BR, KK, CIN], BF16, tag="vt")
                for p in range(KK):
                    ky, kx = p // 3, p % 3
                    y0 = BR * nb
                    r0 = y0 + ky + 1
                    prodbuf = vpool.tile([128, KK, BR, CIN], BF16, tag="pb")
                    for sy in range(3):
                        for sx in range(3):
                            sidx = 3 * sy + sx
                            nc.vector.tensor_tensor(
                                out=prodbuf[:, sidx, :, :],
                                in0=featT5[:, kx + sx, r0 + sy : r0 + sy + BR, :],
                                in1=masks[:, p, sy, sx, y0 : y0 + BR, None]
                                .broadcast_to([128, BR, CIN]),
                                op=ALU.mult)
                    nc.vector.tensor_tensor(out=vt[:, :, p, :], in0=prodbuf[:, 0, :, :],
                                            in1=prodbuf[:, 1, :, :], op=ALU.add)
                    for sidx in range(2, KK):
                        nc.vector.tensor_tensor(out=vt[:, :, p, :], in0=vt[:, :, p, :],
                                                in1=prodbuf[:, sidx, :, :], op=ALU.add)
                val_blk = vblk.tile([128, 5, BR, W], BF16, tag="vb")
                nc.vector.memset(val_blk[64:128, 4, :, :], 0.0)
                for t in range(5):
                    pst = vpsum.tile([128, BR * 128], BF16, tag="bt")
                    for pp in range(2):
                        p = 2 * t + pp
                        if p >= KK:
                            continue
                        for j in range(BR):
                            nc.tensor.transpose(
                                pst[64 * pp : 64 * pp + 64, 128 * j : 128 * (j + 1)],
                                vt[:, j, p, :], identb[:, :])
                    hi = 128 if t < 4 else 64
                    nc.scalar.copy(val_blk[0:hi, t, :, :], pst[0:hi, :])
                oc = och.tile([COUT, BR, W], F32, tag="oc")
                for c2 in range(BR // 4):
                    ps = vpsum.tile([COUT, 512], F32, tag="mm")
                    for t in range(5):
                        nc.tensor.matmul(ps[:, :], w2b[:, t, :],
                                         val_blk[:, t, 4 * c2 : 4 * c2 + 4, :],
                                         start=(t == 0), stop=(t == 4))
                    nc.scalar.activation(oc[:, 4 * c2 : 4 * c2 + 4, :], ps[:, :],
                                         ACTF.Identity, bias=bias2c[:, :])
                nc.sync.dma_start(out=d_out[:, BR * nb : BR * nb + BR, :],
                                  in_=oc[:, :, :])

    nc.compile()
    return nc


def _host_prep(inputs):
    """Build the 8 per-core input maps (numpy marshalling only)."""
    feat = np.ascontiguousarray(inputs["input_feat"], dtype=np.float32)
    inter = np.ascontiguousarray(inputs["inter"], dtype=np.float32)
    fea = np.asarray(inputs["fea"], dtype=np.float32)[:, :, 0, 0]  # [B, 256]
    weight = np.asarray(inputs["weight"], dtype=np.float32)
    bias = np.asarray(inputs["bias"], dtype=np.float32)
    com_w = np.asarray(inputs["com_w"], dtype=np.float32)
    com_b = np.asarray(inputs["com_b"], dtype=np.float32)
    c1_w = np.asarray(inputs["c1_w"], dtype=np.float32)
    c2_w = np.asarray(inputs["c2_w"], dtype=np.float32)

    # fold c2 into the static weight:  weight2[o2, c, p] (parameter prep)
    w_r = weight.reshape(COUT, CIN, KK)
    weight2 = np.einsum("ao,ocp->acp", c2_w, w_r)  # [64, 64, 9]
    w2 = np.zeros((128, 5, 64), np.float32)  # [(c, p-pair), ktile, o2]
    for p in range(KK):
        t, pp = p // 2, p % 2
        w2[64 * pp : 64 * pp + 64, t, :] = weight2[:, :, p].T  # [c, o2]
    bias2 = (c2_w @ bias).reshape(COUT, 1)

    # com_w reordered: channels [dy x9, dx x9, sig x9]; layout [cin, tap, 27]
    perm = list(range(0, 18, 2)) + list(range(1, 18, 2)) + list(range(18, 27))
    comw = np.ascontiguousarray(
        com_w[perm].reshape(27, CIN, KK).transpose(1, 2, 0))  # [CIN, KK, 27]
    comb = com_b[perm].reshape(27, 1).astype(np.float32)

    c1w = np.ascontiguousarray(c1_w.T.reshape(2, 128, COUT).transpose(1, 0, 2))
    ident = np.eye(128, dtype=np.float32)

    in_maps = []
    for i in range(8):
        b, h = i // 2, i % 2
        r0 = NOUT * h
        fpad = np.zeros((CIN, NR, 132), np.float32)
        glo, ghi = r0 - 3, r0 - 3 + NR
        slo, shi = max(0, glo), min(H, ghi)
        fpad[:, slo - glo : shi - glo, 2 : 2 + W] = feat[b, :, slo:shi, :]
        ipad = np.zeros((CIN, NRI, 130), np.float32)
        glo, ghi = r0 - 1, r0 - 1 + NRI
        slo, shi = max(0, glo), min(H, ghi)
        ipad[:, slo - glo : shi - glo, 1 : 1 + W] = inter[b, :, slo:shi, :]
        feac = np.ascontiguousarray(fea[b].reshape(2, 128).T)
        in_maps.append(dict(feat=fpad, inter=ipad, w2=w2, comw=comw, comb=comb,
                            c1w=c1w, fea=feac, bias2=bias2, ident=ident))
    return in_maps


def kernel(**inputs) -> np.ndarray:
    if "nc" not in _CACHED:
        _CACHED["nc"] = _build_nc()
    nc = _CACHED["nc"]
    in_maps = _host_prep(inputs)
    res = run_bass_kernel_spmd(nc, in_maps, core_ids=list(range(8)),
                               **_CACHED.get("run_kwargs", {}))
    _CACHED["last_result"] = res
    out = np.zeros((B, COUT, H, W), np.float32)
    for i in range(8):
        b, h = i // 2, i % 2
        out[b, :, NOUT * h : NOUT * (h + 1), :] = res.results[i]["out"]
    return out



# revision 11
# speedup vs baseline: 1.2830x; 1.2830x over previous
"""Trainium2 Bass kernel for the DCN (modulated deformable conv) layer.

Self-contained: hardcodes all shapes. Shards data-parallel over (batch b x
row-half h) onto 8 NeuronCores; each core computes a [64, 64, 128] slab of
the [4, 64, 128, 128] output.

Per-core algorithm (all position indices x live on SBUF partitions):
  1. om-conv (3x3, PE, bf16): offsets dy,dx + mask logits from `inter`.
  2. PE-transpose om -> omT[x, y, ch]; build bilinear/sigmoid mask planes
     m[p, sy, sx, y][x] on DVE (positions-in-partitions layout).
  3. PE-transpose feat -> featT5[x, dx_shift, row, c] (5 column shifts via
     DMA partition-shifted copies).
  4. Apply: per (tap, 4-row block): 9 contiguous tensor_tensor products
     (mask broadcast along c via a 0-stride free dim) + an 8-op add chain
     on DVE -- 2448 ops total, all operands contiguous [128, 4, 64] bf16.
  5. PE-transpose valT back to val[(c,p)-partitions, (y,x)] and contract with
     the per-sample dynamic weights W2' = c2_w @ (weight * fvec) on PE.
"""

import numpy as np
from contextlib import ExitStack

import concourse.bass as bass
import concourse.bacc as bacc
import concourse.tile as tile
from concourse import mybir
from concourse.bass_utils import run_bass_kernel_spmd

F32 = mybir.dt.float32
BF16 = mybir.dt.bfloat16
ALU = mybir.AluOpType
ACTF = mybir.ActivationFunctionType

B, CIN, COUT, H, W, K = 4, 64, 64, 128, 128, 3
KK = K * K
NOUT = 64          # out rows per core
NR = 69            # feat rows resident per core:  y + ky-1 + sy in [-3, 66)
NRI = 66           # inter rows resident (conv halo 1)
NCH = 16           # position chunks (of 512) per core
CLAMP = 0.999999

_CACHED = {}


def _build_nc():
    nc = bacc.Bacc("TRN2", target_bir_lowering=False)

    # ---- DRAM I/O (per-core views; same program on all 8 cores) ----
    d_feat = nc.dram_tensor("feat", [CIN, NR, 132], F32, kind="ExternalInput")
    d_inter = nc.dram_tensor("inter", [CIN, NRI, 130], F32, kind="ExternalInput")
    d_w2 = nc.dram_tensor("w2", [128, 5, 64], F32, kind="ExternalInput")
    d_comw = nc.dram_tensor("comw", [CIN, KK, 27], F32, kind="ExternalInput")
    d_comb = nc.dram_tensor("comb", [27, 1], F32, kind="ExternalInput")
    d_c1w = nc.dram_tensor("c1w", [128, 2, COUT], F32, kind="ExternalInput")
    d_fea = nc.dram_tensor("fea", [128, 2], F32, kind="ExternalInput")
    d_bias2 = nc.dram_tensor("bias2", [COUT, 1], F32, kind="ExternalInput")
    d_ident = nc.dram_tensor("ident", [128, 128], F32, kind="ExternalInput")
    d_zeros = nc.dram_tensor("zeros", [2, CIN * NR], F32, kind="ExternalInput")
    d_out = nc.dram_tensor("out", [COUT, NOUT, W], F32, kind="ExternalOutput")

    with ExitStack() as ctx:
        tc = ctx.enter_context(tile.TileContext(nc))

        # ---------------- persistent small pool ----------------
        pers = ctx.enter_context(tc.tile_pool(name="pers", bufs=1))
        identb = pers.tile([128, 128], BF16)
        w2cp = pers.tile([128, 5 * 64], F32)
        w2b = pers.tile([128, 5, 64], BF16)
        comwb = pers.tile([CIN, KK, 27], BF16)
        combc = pers.tile([27, 1], F32)
        c1wb = pers.tile([128, 2, COUT], BF16)
        feab = pers.tile([128, 2], BF16)
        bias2c = pers.tile([COUT, 1], F32)
        fvec = pers.tile([128, 1], F32)
        omT = pers.tile([128, NOUT, 27], BF16)
        masks = pers.tile([128, KK, 3, 3, NOUT], BF16)

        identf = pers.tile([128, 128], F32)
        nc.sync.dma_start(out=identf, in_=d_ident[:, :])
        nc.vector.tensor_copy(identb[:, :], identf[:, :])
        nc.sync.dma_start(out=w2cp, in_=d_w2.rearrange("p a b -> p (a b)")[:, :])
        dma_comw = nc.gpsimd.dma_start(out=comwb, in_=d_comw[:, :, :])  # cast f32->bf16
        nc.sync.dma_start(out=combc, in_=d_comb[:, :])
        nc.gpsimd.dma_start(out=c1wb, in_=d_c1w[:, :, :])
        nc.gpsimd.dma_start(out=feab, in_=d_fea[:, :])
        nc.sync.dma_start(out=bias2c, in_=d_bias2[:, :])

        psum_sm = ctx.enter_context(tc.tile_pool(name="psum_sm", bufs=1, space="PSUM"))

        # fvec = c1_w @ fea  -> [64, 1]; replicate to [128, 1]
        ps_fv = psum_sm.tile([COUT, 1], F32)
        for k in range(2):
            nc.tensor.matmul(ps_fv[:, :], c1wb[:, k, :], feab[:, k : k + 1],
                             start=(k == 0), stop=(k == 1))
        nc.scalar.copy(fvec[0:COUT, :], ps_fv[:, :])
        nc.sync.dma_start(out=fvec[COUT:128, :], in_=fvec[0:COUT, :])
        # w2b = (w2cp * fvec[c]) cast to bf16
        nc.vector.tensor_scalar(out=w2b.rearrange("p a b -> p (a b)")[:, :],
                                in0=w2cp[:, :], scalar1=fvec[:, :], scalar2=None,
                                op0=ALU.mult)

        # ---------------- phase 1: om conv + masks ----------------
        with tc.tile_pool(name="omph", bufs=1) as omph:
            interb = omph.tile([CIN, NRI, 130], BF16)
            nc.gpsimd.dma_start(out=interb, in_=d_inter[:, :, :])
            om_sb = omph.tile([27, NOUT, W], BF16)
            with tc.tile_pool(name="ompsum", bufs=2, space="PSUM") as ompsum:
                for n in range(NCH):  # 512-wide position chunks = 4 out rows
                    ps = ompsum.tile([27, 512], F32)
                    y0 = 4 * n
                    for d in range(KK):
                        dy, dx = d // 3, d % 3
                        rhs = interb[:, y0 + dy : y0 + dy + 4, dx : dx + W]
                        nc.tensor.matmul(ps[:, :], comwb[:, d, :], rhs,
                                         start=(d == 0), stop=(d == KK - 1))
                    nc.scalar.activation(om_sb[:, y0 : y0 + 4, :].rearrange("p a b -> p (a b)"),
                                         ps[:, :], ACTF.Identity, bias=combc[:, :])
                # om transpose: [27, 128] row-slices -> omT [128, y, 27]
                for g in range(4):  # 16 rows per psum tile (28-elem slots, 4B-aligned)
                    pst = ompsum.tile([128, 16, 28], BF16)
                    for j in range(16):
                        y = 16 * g + j
                        nc.tensor.transpose(pst[:, j, 0:27],
                                            om_sb[:, y, :], identb[0:27, 0:27])
                    nc.scalar.copy(omT[:, 16 * g : 16 * (g + 1), :], pst[:, :, 0:27])

            # ---- mask build (f32), positions on partitions ----
            mbig = omph.tile([128, 8, KK, NOUT], F32)
            dyT, dxT, sgT, ey, ly, ay, f0, s = [mbig[:, i] for i in range(8)]
            wm_t = omph.tile([128, KK, NOUT], F32, tag="wm")
            wm = wm_t[:, :, :]
            w0_t = omph.tile([128, KK, NOUT], F32, tag="w0")
            w0 = w0_t[:, :, :]
            wp_t = omph.tile([128, KK, NOUT], F32, tag="wp")
            wp = wp_t[:, :, :]
            wys = omph.tile([128, KK, 3, NOUT], F32)
            wxs = omph.tile([128, KK, 3, NOUT], F32)
            # repack dy/dx/sig from omT (ch-minor) into [128, p, y] contiguous
            for dst, lo in [(dyT, 0), (dxT, 9), (sgT, 18)]:
                nc.vector.tensor_copy(dst[:, :, :],
                                      omT[:, :, lo : lo + 9].rearrange("p y c -> p c y"))
            nc.scalar.activation(sgT, sgT, ACTF.Sigmoid)

            for dT, wtile, fold_sig in [(dyT, wys, True), (dxT, wxs, False)]:
                nc.vector.tensor_scalar(out=dT[:, :, :], in0=dT[:, :, :], scalar1=-CLAMP,
                                        scalar2=CLAMP, op0=ALU.max, op1=ALU.min)
                nc.vector.tensor_scalar(out=ey, in0=dT[:, :, :], scalar1=0.0,
                                        scalar2=None, op0=ALU.is_lt)
                nc.vector.tensor_tensor(out=ly, in0=dT[:, :, :], in1=ey,
                                        op=ALU.add)
                nc.vector.tensor_scalar(out=ay, in0=ly, scalar1=-1.0,
                                        scalar2=1.0, op0=ALU.mult, op1=ALU.add)
                nc.vector.tensor_scalar(out=f0, in0=ey, scalar1=-1.0,
                                        scalar2=1.0, op0=ALU.mult, op1=ALU.add)
                nc.vector.tensor_tensor(out=wm, in0=ey, in1=ay,
                                        op=ALU.mult)
                nc.vector.tensor_tensor(out=wp, in0=f0, in1=ly,
                                        op=ALU.mult)
                nc.vector.tensor_tensor(out=s, in0=wm, in1=wp,
                                        op=ALU.add)
                nc.vector.tensor_scalar(out=w0, in0=s, scalar1=-1.0,
                                        scalar2=1.0, op0=ALU.mult, op1=ALU.add)
                for k, wk in enumerate([wm, w0, wp]):
                    if fold_sig:
                        nc.vector.tensor_tensor(out=wtile[:, :, k, :], in0=wk[:, :, :],
                                                in1=sgT, op=ALU.mult)
                    else:
                        nc.vector.tensor_copy(wtile[:, :, k, :], wk[:, :, :])
            # m[p, sy, sx, y] = wys[p, sy, y] * wxs[p, sx, y]
            nc.vector.tensor_tensor(
                out=masks[:, :, :, :, :],
                in0=wys[:, :, :, None, :].broadcast_to([128, KK, 3, 3, NOUT]),
                in1=wxs[:, :, None, :, :].broadcast_to([128, KK, 3, 3, NOUT]),
                op=ALU.mult)

        # ---------------- phase 2: featT5 (rows innermost: [x, shift, c, row]) --
        featT5 = pers.tile([128, 5, CIN, NR], BF16)
        with tc.tile_pool(name="featph", bufs=1) as featph:
            featb = featph.tile([CIN, NR, 132], BF16)
            nc.gpsimd.dma_start(out=featb, in_=d_feat[:, :, :])
            with tc.tile_pool(name="ftpsum", bufs=4, space="PSUM") as ftpsum:
                for g in range((NR + 7) // 8):  # 8 rows per psum tile
                    rows = range(8 * g, min(8 * g + 8, NR))
                    pst = ftpsum.tile([128, 8 * CIN], BF16)
                    for j, r in enumerate(rows):
                        nc.tensor.transpose(pst[:, CIN * j : CIN * (j + 1)],
                                            featb[:, r, 2 : 2 + 128], identb[0:CIN, 0:CIN])
                    nc.scalar.copy(
                        featT5[:, 2, :, 8 * g : 8 * g + len(rows)],
                        pst[:, : len(rows) * CIN].rearrange("p (j c) -> p c j", j=len(rows)))
        # shifted copies: featT5[:, 2+d, :, :][x] = featT0[x + d]
        flat = featT5.rearrange("p a b c -> p a (b c)")
        for dlt, di in [(-2, 0), (-1, 1), (1, 3), (2, 4)]:
            if dlt > 0:
                nc.sync.dma_start(out=flat[0 : 128 - dlt, di, :], in_=flat[dlt:128, 2, :])
            else:
                nc.sync.dma_start(out=flat[-dlt : 128, di, :], in_=flat[0 : 128 + dlt, 2, :])
        # zero only the edge partitions the shifts leave stale (DMA from a
        # zero-filled DRAM strip; Pool-engine memsets can't address these
        # partition ranges)
        nc.gpsimd.dma_start(out=flat[0:2, 0, :], in_=d_zeros[:, :])
        nc.gpsimd.dma_start(out=flat[0:1, 1, :], in_=d_zeros[0:1, :])
        nc.gpsimd.dma_start(out=flat[127:128, 3, :], in_=d_zeros[0:1, :])
        nc.gpsimd.dma_start(out=flat[126:128, 4, :], in_=d_zeros[:, :])

        # ---------------- phase 3: apply + back-transpose + einsum ----------------
        with (
            tc.tile_pool(name="vpool", bufs=2) as vpool,
            tc.tile_pool(name="papool", bufs=1) as papool,
            tc.tile_pool(name="vblk", bufs=1) as vblk,
            tc.tile_pool(name="och", bufs=2) as och,
            tc.tile_pool(name="vpsum", bufs=2, space="PSUM") as vpsum,
        ):
            BR = 16  # out rows per apply block
            for nb in range(NOUT // BR):
                y0 = BR * nb
                # vt[x, t, pp*64+c, y]: tap-pairs adjacent in the 128 dim so
                # each back-transpose handles two taps at once
                vt = vpool.tile([128, 5, 128, BR], BF16, tag="vt")
                nc.gpsimd.memset(vt[:, 4, 64:128, :], 0.0)
                for p in range(KK):
                    ky, kx = p // 3, p % 3
                    r0 = y0 + ky + 1
                    t, pp = p // 2, p % 2
                    prodbuf = papool.tile([128, KK, CIN, BR], BF16, tag="pb")
                    for sy in range(3):
                        # one op per sy: free dims (sx, c, y), y packed last;
                        # mask broadcast over c is a 0-stride middle dim -> 2x
                        nc.vector.tensor_tensor(
                            out=prodbuf[:, 3 * sy : 3 * sy + 3, :, :],
                            in0=featT5[:, kx : kx + 3, :, r0 + sy : r0 + sy + BR],
                            in1=masks[:, p, sy, :, None, y0 : y0 + BR]
                            .broadcast_to([128, 3, CIN, BR]),
                            op=ALU.mult)
                    acc = vt[:, t, 64 * pp : 64 * pp + 64, :]
                    s1 = papool.tile([128, 4, CIN, BR], BF16, tag="s1")
                    nc.vector.tensor_tensor(out=s1[:, :, :, :], in0=prodbuf[:, 0:4],
                                            in1=prodbuf[:, 4:8], op=ALU.add)
                    nc.vector.tensor_tensor(out=s1[:, 0:2], in0=s1[:, 0:2],
                                            in1=s1[:, 2:4], op=ALU.add)
                    nc.vector.tensor_tensor(out=acc, in0=s1[:, 0], in1=s1[:, 1],
                                            op=ALU.add)
                    nc.vector.tensor_tensor(out=acc, in0=acc, in1=prodbuf[:, 8],
                                            op=ALU.add)
                val_blk = vblk.tile([128, 5, BR, W], BF16, tag="vb")
                for t in range(5):
                    pst = vpsum.tile([128, BR * 128], BF16, tag="bt")
                    for j in range(BR):
                        nc.tensor.transpose(pst[:, 128 * j : 128 * (j + 1)],
                                            vt[:, t, :, j], identb[:, :])
                    nc.scalar.copy(val_blk[:, t, :, :], pst[:, :])
                oc = och.tile([COUT, BR, W], F32, tag="oc")
                for c2 in range(BR // 4):
                    ps = vpsum.tile([COUT, 512], F32, tag="mm")
                    for t in range(5):
                        nc.tensor.matmul(ps[:, :], w2b[:, t, :],
                                         val_blk[:, t, 4 * c2 : 4 * c2 + 4, :],
                                         start=(t == 0), stop=(t == 4))
                    nc.scalar.activation(oc[:, 4 * c2 : 4 * c2 + 4, :], ps[:, :],
                                         ACTF.Identity, bias=bias2c[:, :])
                nc.sync.dma_start(out=d_out[:, BR * nb : BR * nb + BR, :],
                                  in_=oc[:, :, :])

    nc.compile()
    return nc


def _host_prep(inputs):
    """Build the 8 per-core input maps (numpy marshalling only)."""
    feat = np.ascontiguousarray(inputs["input_feat"], dtype=np.float32)
    inter = np.ascontiguousarray(inputs["inter"], dtype=np.float32)
    fea = np.asarray(inputs["fea"], dtype=np.float32)[:, :, 0, 0]  # [B, 256]
    weight = np.asarray(inputs["weight"], dtype=np.float32)
    bias = np.asarray(inputs["bias"], dtype=np.float32)
    com_w = np.asarray(inputs["com_w"], dtype=np.float32)
    com_b = np.asarray(inputs["com_b"], dtype=np.float32)
    c1_w = np.asarray(inputs["c1_w"], dtype=np.float32)
    c2_w = np.asarray(inputs["c2_w"], dtype=np.float32)

    # fold c2 into the static weight:  weight2[o2, c, p] (parameter prep)
    w_r = weight.reshape(COUT, CIN, KK)
    weight2 = np.einsum("ao,ocp->acp", c2_w, w_r)  # [64, 64, 9]
    w2 = np.zeros((128, 5, 64), np.float32)  # [(c, p-pair), ktile, o2]
    for p in range(KK):
        t, pp = p // 2, p % 2
        w2[64 * pp : 64 * pp + 64, t, :] = weight2[:, :, p].T  # [c, o2]
    bias2 = (c2_w @ bias).reshape(COUT, 1)

    # com_w reordered: channels [dy x9, dx x9, sig x9]; layout [cin, tap, 27]
    perm = list(range(0, 18, 2)) + list(range(1, 18, 2)) + list(range(18, 27))
    comw = np.ascontiguousarray(
        com_w[perm].reshape(27, CIN, KK).transpose(1, 2, 0))  # [CIN, KK, 27]
    comb = com_b[perm].reshape(27, 1).astype(np.float32)

    c1w = np.ascontiguousarray(c1_w.T.reshape(2, 128, COUT).transpose(1, 0, 2))
    ident = np.eye(128, dtype=np.float32)

    in_maps = []
    for i in range(8):
        b, h = i // 2, i % 2
        r0 = NOUT * h
        fpad = np.zeros((CIN, NR, 132), np.float32)
        glo, ghi = r0 - 3, r0 - 3 + NR
        slo, shi = max(0, glo), min(H, ghi)
        fpad[:, slo - glo : shi - glo, 2 : 2 + W] = feat[b, :, slo:shi, :]
        ipad = np.zeros((CIN, NRI, 130), np.float32)
        glo, ghi = r0 - 1, r0 - 1 + NRI
        slo, shi = max(0, glo), min(H, ghi)
        ipad[:, slo - glo : shi - glo, 1 : 1 + W] = inter[b, :, slo:shi, :]
        feac = np.ascontiguousarray(fea[b].reshape(2, 128).T)
        in_maps.append(dict(feat=fpad, inter=ipad, w2=w2, comw=comw, comb=comb,
                            c1w=c1w, fea=feac, bias2=bias2, ident=ident,
                            zeros=np.zeros((2, CIN * NR), np.float32)))
    return in_maps


def kernel(**inputs) -> np.ndarray:
    if "nc" not in _CACHED:
        _CACHED["nc"] = _build_nc()
    nc = _CACHED["nc"]
    in_maps = _host_prep(inputs)
    res = run_bass_kernel_spmd(nc, in_maps, core_ids=list(range(8)),
                               **_CACHED.get("run_kwargs", {}))
    _CACHED["last_result"] = res
    out = np.zeros((B, COUT, H, W), np.float32)
    for i in range(8):
        b, h = i // 2, i % 2
        out[b, :, NOUT * h : NOUT * (h + 1), :] = res.results[i]["out"]
    return out



# revision 14
# speedup vs baseline: 1.4875x; 1.1594x over previous
"""Trainium2 Bass kernel for the DCN (modulated deformable conv) layer.

Self-contained: hardcodes all shapes. Shards data-parallel over (batch b x
row-half h) onto 8 NeuronCores; each core computes a [64, 64, 128] slab of
the [4, 64, 128, 128] output.

Per-core algorithm (all position indices x live on SBUF partitions):
  1. om-conv (3x3, PE, bf16): offsets dy,dx + mask logits from `inter`.
  2. PE-transpose om -> omT[x, y, ch]; build bilinear/sigmoid mask planes
     m[p, sy, sx, y][x] on DVE (positions-in-partitions layout).
  3. PE-transpose feat -> featT5[x, dx_shift, row, c] (5 column shifts via
     DMA partition-shifted copies).
  4. Apply: per (tap, 4-row block): 9 contiguous tensor_tensor products
     (mask broadcast along c via a 0-stride free dim) + an 8-op add chain
     on DVE -- 2448 ops total, all operands contiguous [128, 4, 64] bf16.
  5. PE-transpose valT back to val[(c,p)-partitions, (y,x)] and contract with
     the per-sample dynamic weights W2' = c2_w @ (weight * fvec) on PE.
"""

import numpy as np
from contextlib import ExitStack

import concourse.bass as bass
import concourse.bacc as bacc
import concourse.tile as tile
from concourse import mybir
from concourse.bass_utils import run_bass_kernel_spmd

F32 = mybir.dt.float32
BF16 = mybir.dt.bfloat16
ALU = mybir.AluOpType
ACTF = mybir.ActivationFunctionType

B, CIN, COUT, H, W, K = 4, 64, 64, 128, 128, 3
KK = K * K
NOUT = 64          # out rows per core
NR = 69            # feat rows resident per core:  y + ky-1 + sy in [-3, 66)
NRI = 66           # inter rows resident (conv halo 1)
NCH = 16           # position chunks (of 512) per core
CLAMP = 0.999999

_CACHED = {}


def _build_nc():
    nc = bacc.Bacc("TRN2", target_bir_lowering=False)

    # ---- DRAM I/O (per-core views; same program on all 8 cores) ----
    d_feat = nc.dram_tensor("feat", [CIN, NR, 132], F32, kind="ExternalInput")
    d_inter = nc.dram_tensor("inter", [CIN, NRI, 130], F32, kind="ExternalInput")
    d_w2 = nc.dram_tensor("w2", [128, 5, 64], F32, kind="ExternalInput")
    d_comw = nc.dram_tensor("comw", [CIN, KK, 27], F32, kind="ExternalInput")
    d_comb = nc.dram_tensor("comb", [27, 1], F32, kind="ExternalInput")
    d_c1w = nc.dram_tensor("c1w", [128, 2, COUT], F32, kind="ExternalInput")
    d_fea = nc.dram_tensor("fea", [128, 2], F32, kind="ExternalInput")
    d_bias2 = nc.dram_tensor("bias2", [COUT, 1], F32, kind="ExternalInput")
    d_ident = nc.dram_tensor("ident", [128, 128], F32, kind="ExternalInput")
    d_zeros = nc.dram_tensor("zeros", [2, CIN * NR], F32, kind="ExternalInput")
    d_out = nc.dram_tensor("out", [COUT, NOUT, W], F32, kind="ExternalOutput")

    with ExitStack() as ctx:
        tc = ctx.enter_context(tile.TileContext(nc))

        # ---------------- persistent small pool ----------------
        pers = ctx.enter_context(tc.tile_pool(name="pers", bufs=1))
        identb = pers.tile([128, 128], BF16)
        w2cp = pers.tile([128, 5 * 64], F32)
        w2b = pers.tile([128, 5, 64], BF16)
        comwb = pers.tile([CIN, KK, 27], BF16)
        combc = pers.tile([27, 1], F32)
        c1wb = pers.tile([128, 2, COUT], BF16)
        feab = pers.tile([128, 2], BF16)
        bias2c = pers.tile([COUT, 1], F32)
        fvec = pers.tile([128, 1], F32)

        identf = pers.tile([128, 128], F32)
        nc.sync.dma_start(out=identf, in_=d_ident[:, :])
        nc.vector.tensor_copy(identb[:, :], identf[:, :])
        nc.sync.dma_start(out=w2cp, in_=d_w2.rearrange("p a b -> p (a b)")[:, :])
        dma_comw = nc.gpsimd.dma_start(out=comwb, in_=d_comw[:, :, :])  # cast f32->bf16
        nc.sync.dma_start(out=combc, in_=d_comb[:, :])
        nc.gpsimd.dma_start(out=c1wb, in_=d_c1w[:, :, :])
        nc.gpsimd.dma_start(out=feab, in_=d_fea[:, :])
        nc.sync.dma_start(out=bias2c, in_=d_bias2[:, :])

        with tc.tile_pool(name="psum_sm", bufs=1, space="PSUM") as psum_sm:
            # fvec = c1_w @ fea  -> [64, 1]; replicate to [128, 1]
            ps_fv = psum_sm.tile([COUT, 1], F32)
            for k in range(2):
                nc.tensor.matmul(ps_fv[:, :], c1wb[:, k, :], feab[:, k : k + 1],
                                 start=(k == 0), stop=(k == 1))
            nc.scalar.copy(fvec[0:COUT, :], ps_fv[:, :])
        nc.sync.dma_start(out=fvec[COUT:128, :], in_=fvec[0:COUT, :])
        # w2b = (w2cp * fvec[c]) cast to bf16
        nc.vector.tensor_scalar(out=w2b.rearrange("p a b -> p (a b)")[:, :],
                                in0=w2cp[:, :], scalar1=fvec[:, :], scalar2=None,
                                op0=ALU.mult)

        # ---------------- phase A: featT5 (rows innermost: [x, shift, c, row]) --
        featT5 = pers.tile([128, 5, CIN, NR], BF16)
        interb = pers.tile([CIN, NRI, 130], BF16)
        nc.gpsimd.dma_start(out=interb, in_=d_inter[:, :, :])
        with tc.tile_pool(name="featph", bufs=1) as featph:
            featb = featph.tile([CIN, NR, 132], BF16)
            nc.gpsimd.dma_start(out=featb, in_=d_feat[:, :, :])
            with tc.tile_pool(name="ftpsum", bufs=4, space="PSUM") as ftpsum:
                for g in range((NR + 7) // 8):  # 8 rows per psum tile
                    rows = range(8 * g, min(8 * g + 8, NR))
                    pst = ftpsum.tile([128, 8 * CIN], BF16)
                    for j, r in enumerate(rows):
                        nc.tensor.transpose(pst[:, CIN * j : CIN * (j + 1)],
                                            featb[:, r, 2 : 2 + 128], identb[0:CIN, 0:CIN])
                    nc.scalar.copy(
                        featT5[:, 2, :, 8 * g : 8 * g + len(rows)],
                        pst[:, : len(rows) * CIN].rearrange("p (j c) -> p c j", j=len(rows)))
        # shifted copies: featT5[:, 2+d, :, :][x] = featT0[x + d]
        flat = featT5.rearrange("p a b c -> p a (b c)")
        for dlt, di in [(-2, 0), (-1, 1), (1, 3), (2, 4)]:
            if dlt > 0:
                nc.sync.dma_start(out=flat[0 : 128 - dlt, di, :], in_=flat[dlt:128, 2, :])
            else:
                nc.sync.dma_start(out=flat[-dlt : 128, di, :], in_=flat[0 : 128 + dlt, 2, :])
        # zero the edge partitions the shifts leave stale (Pool-engine memsets
        # cannot address these partition ranges; DMA from a zero DRAM strip)
        nc.gpsimd.dma_start(out=flat[0:2, 0, :], in_=d_zeros[:, :])
        nc.gpsimd.dma_start(out=flat[0:1, 1, :], in_=d_zeros[0:1, :])
        nc.gpsimd.dma_start(out=flat[127:128, 3, :], in_=d_zeros[0:1, :])
        nc.gpsimd.dma_start(out=flat[126:128, 4, :], in_=d_zeros[:, :])

        # ------- pipelined per-16-row blocks: om conv -> masks -> apply -------
        BR = 16
        with (
            tc.tile_pool(name="omph", bufs=2) as omph,
            tc.tile_pool(name="mtmp", bufs=1) as mtmp,
            tc.tile_pool(name="mout", bufs=2) as mout,
            tc.tile_pool(name="vpool", bufs=2) as vpool,
            tc.tile_pool(name="papool", bufs=1) as papool,
            tc.tile_pool(name="vblk", bufs=1) as vblk,
            tc.tile_pool(name="och", bufs=2) as och,
            tc.tile_pool(name="ompsum", bufs=2, space="PSUM") as ompsum,
            tc.tile_pool(name="btps", bufs=2, space="PSUM") as btps,
            tc.tile_pool(name="mmps", bufs=2, space="PSUM") as mmps,
        ):
            for g in range(NOUT // BR):
                y0 = BR * g
                # --- om conv rows y0..y0+BR ---
                om_sb = omph.tile([27, BR, W], BF16, tag="omsb")
                for nch in range(BR // 4):
                    ps = ompsum.tile([27, 512], F32, tag="ps")
                    yy = y0 + 4 * nch
                    for d in range(KK):
                        dy, dx = d // 3, d % 3
                        rhs = interb[:, yy + dy : yy + dy + 4, dx : dx + W]
                        nc.tensor.matmul(ps[:, :], comwb[:, d, :], rhs,
                                         start=(d == 0), stop=(d == KK - 1))
                    nc.scalar.activation(
                        om_sb[:, 4 * nch : 4 * nch + 4, :].rearrange("p a b -> p (a b)"),
                        ps[:, :], ACTF.Identity, bias=combc[:, :])
                omTg = omph.tile([128, BR, 27], BF16, tag="omT")
                pst = ompsum.tile([128, BR, 28], BF16, tag="pst")
                for j in range(BR):
                    nc.tensor.transpose(pst[:, j, 0:27], om_sb[:, j, :],
                                        identb[0:27, 0:27])
                nc.scalar.copy(omTg[:, :, :], pst[:, :, 0:27])

                # --- mask build for this block (positions on partitions) ---
                mbig = mtmp.tile([128, 8, KK, BR], F32, tag="mbig")
                dyT, dxT, sgT, ey, ly, ay, f0, s = [mbig[:, i] for i in range(8)]
                wm_t = mtmp.tile([128, KK, BR], F32, tag="wm")
                wm = wm_t[:, :, :]
                w0_t = mtmp.tile([128, KK, BR], F32, tag="w0")
                w0 = w0_t[:, :, :]
                wp_t = mtmp.tile([128, KK, BR], F32, tag="wp")
                wp = wp_t[:, :, :]
                wys = mtmp.tile([128, KK, 3, BR], F32, tag="wys")
                wxs = mtmp.tile([128, KK, 3, BR], F32, tag="wxs")
                masks = mout.tile([128, KK, 3, 3, BR], BF16, tag="masks")
                for dst, lo in [(dyT, 0), (dxT, 9), (sgT, 18)]:
                    nc.vector.tensor_copy(dst[:, :, :],
                                          omTg[:, :, lo : lo + 9].rearrange("p y c -> p c y"))
                nc.scalar.activation(sgT, sgT, ACTF.Sigmoid)
                for dT, wtile, fold_sig in [(dyT, wys, True), (dxT, wxs, False)]:
                    nc.vector.tensor_scalar(out=dT[:, :, :], in0=dT[:, :, :], scalar1=-CLAMP,
                                            scalar2=CLAMP, op0=ALU.max, op1=ALU.min)
                    nc.vector.tensor_scalar(out=ey, in0=dT[:, :, :], scalar1=0.0,
                                            scalar2=None, op0=ALU.is_lt)
                    nc.vector.tensor_tensor(out=ly, in0=dT[:, :, :], in1=ey, op=ALU.add)
                    nc.vector.tensor_scalar(out=ay, in0=ly, scalar1=-1.0,
                                            scalar2=1.0, op0=ALU.mult, op1=ALU.add)
                    nc.vector.tensor_scalar(out=f0, in0=ey, scalar1=-1.0,
                                            scalar2=1.0, op0=ALU.mult, op1=ALU.add)
                    nc.vector.tensor_tensor(out=wm, in0=ey, in1=ay, op=ALU.mult)
                    nc.vector.tensor_tensor(out=wp, in0=f0, in1=ly, op=ALU.mult)
                    nc.vector.tensor_tensor(out=s, in0=wm, in1=wp, op=ALU.add)
                    nc.vector.tensor_scalar(out=w0, in0=s, scalar1=-1.0,
                                            scalar2=1.0, op0=ALU.mult, op1=ALU.add)
                    for k, wk in enumerate([wm, w0, wp]):
                        if fold_sig:
                            nc.vector.tensor_tensor(out=wtile[:, :, k, :], in0=wk[:, :, :],
                                                    in1=sgT, op=ALU.mult)
                        else:
                            nc.vector.tensor_copy(wtile[:, :, k, :], wk[:, :, :])
                # m[p, sy, sx, y] = wys[p, sy, y] * wxs[p, sx, y]
                nc.vector.tensor_tensor(
                    out=masks[:, :, :, :, :],
                    in0=wys[:, :, :, None, :].broadcast_to([128, KK, 3, 3, BR]),
                    in1=wxs[:, :, None, :, :].broadcast_to([128, KK, 3, 3, BR]),
                    op=ALU.mult)

                # --- apply: products (2x DVE), tree adds, paired back-transpose,
                #     contraction with W2' ---
                vt = vpool.tile([128, 5, 128, BR], BF16, tag="vt")
                nc.gpsimd.memset(vt[:, 4, 64:128, :], 0.0)
                for p in range(KK):
                    ky, kx = p // 3, p % 3
                    r0 = y0 + ky + 1
                    t, pp = p // 2, p % 2
                    prodbuf = papool.tile([128, KK, CIN, BR], BF16, tag="pb")
                    for sy in range(3):
                        # free dims (sx, c, y): y packed last; mask c-broadcast
                        # is a 0-stride middle dim -> DVE 2x mode
                        nc.vector.tensor_tensor(
                            out=prodbuf[:, 3 * sy : 3 * sy + 3, :, :],
                            in0=featT5[:, kx : kx + 3, :, r0 + sy : r0 + sy + BR],
                            in1=masks[:, p, sy, :, None, :]
                            .broadcast_to([128, 3, CIN, BR]),
                            op=ALU.mult)
                    acc = vt[:, t, 64 * pp : 64 * pp + 64, :]
                    s1 = papool.tile([128, 4, CIN, BR], BF16, tag="s1")
                    nc.vector.tensor_tensor(out=s1[:, :, :, :], in0=prodbuf[:, 0:4],
                                            in1=prodbuf[:, 4:8], op=ALU.add)
                    nc.vector.tensor_tensor(out=s1[:, 0:2], in0=s1[:, 0:2],
                                            in1=s1[:, 2:4], op=ALU.add)
                    nc.vector.tensor_tensor(out=acc, in0=s1[:, 0], in1=s1[:, 1],
                                            op=ALU.add)
                    nc.vector.tensor_tensor(out=acc, in0=acc, in1=prodbuf[:, 8],
                                            op=ALU.add)
                val_blk = vblk.tile([128, 5, BR, W], BF16, tag="vb")
                for t in range(5):
                    for hh in range(2):
                        pst2 = btps.tile([128, 8 * 128], BF16, tag="bt")
                        for j8 in range(8):
                            j = 8 * hh + j8
                            nc.tensor.transpose(pst2[:, 128 * j8 : 128 * (j8 + 1)],
                                                vt[:, t, :, j], identb[:, :])
                        nc.scalar.copy(val_blk[:, t, 8 * hh : 8 * hh + 8, :], pst2[:, :])
                oc = och.tile([COUT, BR, W], F32, tag="oc")
                for c2 in range(BR // 4):
                    ps2 = mmps.tile([COUT, 512], F32, tag="mm")
                    for t in range(5):
                        nc.tensor.matmul(ps2[:, :], w2b[:, t, :],
                                         val_blk[:, t, 4 * c2 : 4 * c2 + 4, :],
                                         start=(t == 0), stop=(t == 4))
                    nc.scalar.activation(oc[:, 4 * c2 : 4 * c2 + 4, :], ps2[:, :],
                                         ACTF.Identity, bias=bias2c[:, :])
                nc.sync.dma_start(out=d_out[:, y0 : y0 + BR, :], in_=oc[:, :, :])

    nc.compile()
    return nc


def _host_prep(inputs):
    """Build the 8 per-core input maps (numpy marshalling only)."""
    feat = np.ascontiguousarray(inputs["input_feat"], dtype=np.float32)
    inter = np.ascontiguousarray(inputs["inter"], dtype=np.float32)
    fea = np.asarray(inputs["fea"], dtype=np.float32)[:, :, 0, 0]  # [B, 256]
    weight = np.asarray(inputs["weight"], dtype=np.float32)
    bias = np.asarray(inputs["bias"], dtype=np.float32)
    com_w = np.asarray(inputs["com_w"], dtype=np.float32)
    com_b = np.asarray(inputs["com_b"], dtype=np.float32)
    c1_w = np.asarray(inputs["c1_w"], dtype=np.float32)
    c2_w = np.asarray(inputs["c2_w"], dtype=np.float32)

    # fold c2 into the static weight:  weight2[o2, c, p] (parameter prep)
    w_r = weight.reshape(COUT, CIN, KK)
    weight2 = np.einsum("ao,ocp->acp", c2_w, w_r)  # [64, 64, 9]
    w2 = np.zeros((128, 5, 64), np.float32)  # [(c, p-pair), ktile, o2]
    for p in range(KK):
        t, pp = p // 2, p % 2
        w2[64 * pp : 64 * pp + 64, t, :] = weight2[:, :, p].T  # [c, o2]
    bias2 = (c2_w @ bias).reshape(COUT, 1)

    # com_w reordered: channels [dy x9, dx x9, sig x9]; layout [cin, tap, 27]
    perm = list(range(0, 18, 2)) + list(range(1, 18, 2)) + list(range(18, 27))
    comw = np.ascontiguousarray(
        com_w[perm].reshape(27, CIN, KK).transpose(1, 2, 0))  # [CIN, KK, 27]
    comb = com_b[perm].reshape(27, 1).astype(np.float32)

    c1w = np.ascontiguousarray(c1_w.T.reshape(2, 128, COUT).transpose(1, 0, 2))
    ident = np.eye(128, dtype=np.float32)

    in_maps = []
    for i in range(8):
        b, h = i // 2, i % 2
        r0 = NOUT * h
        fpad = np.zeros((CIN, NR, 132), np.float32)
        glo, ghi = r0 - 3, r0 - 3 + NR
        slo, shi = max(0, glo), min(H, ghi)
        fpad[:, slo - glo : shi - glo, 2 : 2 + W] = feat[b, :, slo:shi, :]
        ipad = np.zeros((CIN, NRI, 130), np.float32)
        glo, ghi = r0 - 1, r0 - 1 + NRI
        slo, shi = max(0, glo), min(H, ghi)
        ipad[:, slo - glo : shi - glo, 1 : 1 + W] = inter[b, :, slo:shi, :]
        feac = np.ascontiguousarray(fea[b].reshape(2, 128).T)
        in_maps.append(dict(feat=fpad, inter=ipad, w2=w2, comw=comw, comb=comb,
                            c1w=c1w, fea=feac, bias2=bias2, ident=ident,
                            zeros=np.zeros((2, CIN * NR), np.float32)))
    return in_maps


def kernel(**inputs) -> np.ndarray:
    if "nc" not in _CACHED:
        _CACHED["nc"] = _build_nc()
    nc = _CACHED["nc"]
    in_maps = _host_prep(inputs)
    res = run_bass_kernel_spmd(nc, in_maps, core_ids=list(range(8)),
                               **_CACHED.get("run_kwargs", {}))
    _CACHED["last_result"] = res
    out = np.zeros((B, COUT, H, W), np.float32)
    for i in range(8):
        b, h = i // 2, i % 2
        out[b, :, NOUT * h : NOUT * (h + 1), :] = res.results[i]["out"]
    return out



# revision 22
# speedup vs baseline: 1.6147x; 1.0855x over previous
"""Trainium2 Bass kernel for the DCN (modulated deformable conv) layer.

Self-contained: hardcodes all shapes. Shards data-parallel over (batch b x
row-half h) onto 8 NeuronCores; each core computes a [64, 64, 128] slab of
the [4, 64, 128, 128] output.

Per-core algorithm (all position indices x live on SBUF partitions):
  1. om-conv (3x3, PE, bf16): offsets dy,dx + mask logits from `inter`.
  2. PE-transpose om -> omT[x, y, ch]; build bilinear/sigmoid mask planes
     m[p, sy, sx, y][x] on DVE (positions-in-partitions layout).
  3. PE-transpose feat -> featT5[x, dx_shift, row, c] (5 column shifts via
     DMA partition-shifted copies).
  4. Apply: per (tap, 4-row block): 9 contiguous tensor_tensor products
     (mask broadcast along c via a 0-stride free dim) + an 8-op add chain
     on DVE -- 2448 ops total, all operands contiguous [128, 4, 64] bf16.
  5. PE-transpose valT back to val[(c,p)-partitions, (y,x)] and contract with
     the per-sample dynamic weights W2' = c2_w @ (weight * fvec) on PE.
"""

import numpy as np
from contextlib import ExitStack

import concourse.bass as bass
import concourse.bacc as bacc
import concourse.tile as tile
from concourse import mybir
from concourse.bass_utils import run_bass_kernel_spmd

F32 = mybir.dt.float32
BF16 = mybir.dt.bfloat16
ALU = mybir.AluOpType
ACTF = mybir.ActivationFunctionType

B, CIN, COUT, H, W, K = 4, 64, 64, 128, 128, 3
KK = K * K
NOUT = 64          # out rows per core
NR = 69            # feat rows resident per core:  y + ky-1 + sy in [-3, 66)
NRI = 66           # inter rows resident (conv halo 1)
NCH = 16           # position chunks (of 512) per core
CLAMP = 0.999999

_CACHED = {}


def _build_nc():
    nc = bacc.Bacc("TRN2", target_bir_lowering=False)

    # ---- DRAM I/O (per-core views; same program on all 8 cores) ----
    d_feat = nc.dram_tensor("feat", [CIN, NR, 132], F32, kind="ExternalInput")
    d_inter = nc.dram_tensor("inter", [CIN, NRI, 130], F32, kind="ExternalInput")
    d_w2 = nc.dram_tensor("w2", [128, 5, 64], F32, kind="ExternalInput")
    d_comw = nc.dram_tensor("comw", [CIN, KK, 27], F32, kind="ExternalInput")
    d_comb = nc.dram_tensor("comb", [27, 1], F32, kind="ExternalInput")
    d_c1w = nc.dram_tensor("c1w", [128, 2, COUT], F32, kind="ExternalInput")
    d_fea = nc.dram_tensor("fea", [128, 2], F32, kind="ExternalInput")
    d_bias2 = nc.dram_tensor("bias2", [COUT, 1], F32, kind="ExternalInput")
    d_ident = nc.dram_tensor("ident", [128, 128], F32, kind="ExternalInput")
    d_out = nc.dram_tensor("out", [COUT, NOUT, W], F32, kind="ExternalOutput")

    with ExitStack() as ctx:
        tc = ctx.enter_context(tile.TileContext(nc))

        # ---------------- persistent small pool ----------------
        pers = ctx.enter_context(tc.tile_pool(name="pers", bufs=1))
        identb = pers.tile([128, 128], BF16)
        w2cp = pers.tile([128, 5 * 64], F32)
        w2b = pers.tile([128, 5, 64], BF16)
        comwb = pers.tile([CIN, KK, 27], BF16)
        combc = pers.tile([27, 1], F32)
        c1wb = pers.tile([128, 2, COUT], BF16)
        feab = pers.tile([128, 2], BF16)
        bias2c = pers.tile([COUT, 1], F32)
        fvec = pers.tile([128, 1], F32)

        identf = pers.tile([128, 128], F32)
        nc.sync.dma_start(out=identf, in_=d_ident[:, :])
        nc.vector.tensor_copy(identb[:, :], identf[:, :])
        nc.sync.dma_start(out=w2cp, in_=d_w2.rearrange("p a b -> p (a b)")[:, :])
        nc.sync.dma_start(out=combc, in_=d_comb[:, :])
        nc.sync.dma_start(out=bias2c, in_=d_bias2[:, :])

        # ---------------- phase A setup: featT5 written directly in 5 shifted
        # planes by transposing padded cols di:di+128 (no SBUF shift DMAs) ----
        featT5 = pers.tile([128, 5, CIN, NR], BF16)
        interb = pers.tile([CIN, NRI, 130], BF16)
        featb = pers.tile([CIN, NR, 132], BF16)
        nc.gpsimd.dma_start(out=featb, in_=d_feat[:, :, :])
        nc.gpsimd.dma_start(out=interb, in_=d_inter[:, :, :])
        nc.gpsimd.dma_start(out=comwb, in_=d_comw[:, :, :])  # cast f32->bf16
        nc.gpsimd.dma_start(out=c1wb, in_=d_c1w[:, :, :])
        nc.gpsimd.dma_start(out=feab, in_=d_fea[:, :])

        # ------- pipelined per-16-row blocks: om conv -> masks -> apply -------
        BR = 16
        with (
            tc.tile_pool(name="omph", bufs=2) as omph,
            tc.tile_pool(name="mtmp", bufs=1) as mtmp,
            tc.tile_pool(name="mout", bufs=2) as mout,
            tc.tile_pool(name="vpool", bufs=2) as vpool,
            tc.tile_pool(name="papool", bufs=1) as papool,
            tc.tile_pool(name="vblk", bufs=1) as vblk,
            tc.tile_pool(name="och", bufs=1) as och,
            tc.tile_pool(name="ompsum", bufs=1, space="PSUM") as ompsum,
            tc.tile_pool(name="ftps", bufs=2, space="PSUM") as ftps,
            tc.tile_pool(name="btps", bufs=2, space="PSUM") as btps,
            tc.tile_pool(name="mmps", bufs=2, space="PSUM") as mmps,
        ):
            def emit_ft(groups):
                # transpose feat rows into all 5 shifted planes: plane di takes
                # padded cols di:di+128 (host zero-pad covers the edges)
                for g9 in groups:
                    rows = range(8 * g9, min(8 * g9 + 8, NR))
                    for di in range(5):
                        pstf = ftps.tile([128, 8 * CIN], BF16, tag="ftp")
                        for j, r in enumerate(rows):
                            nc.tensor.transpose(pstf[:, CIN * j : CIN * (j + 1)],
                                                featb[:, r, di : di + 128],
                                                identb[0:CIN, 0:CIN])
                        nc.scalar.copy(
                            featT5[:, di, :, 8 * g9 : 8 * g9 + len(rows)],
                            pstf[:, : len(rows) * CIN]
                            .rearrange("p (j c) -> p c j", j=len(rows)))

            def emit_om(g):
                y0 = BR * g
                om_sb = omph.tile([27, BR, W], BF16, tag="omsb")
                for nch in range(BR // 4):
                    ps = ompsum.tile([27, 512], F32, tag="ps")
                    yy = y0 + 4 * nch
                    for d in range(KK):
                        dy, dx = d // 3, d % 3
                        rhs = interb[:, yy + dy : yy + dy + 4, dx : dx + W]
                        nc.tensor.matmul(ps[:, :], comwb[:, d, :], rhs,
                                         start=(d == 0), stop=(d == KK - 1))
                    nc.scalar.activation(
                        om_sb[:, 4 * nch : 4 * nch + 4, :].rearrange("p a b -> p (a b)"),
                        ps[:, :], ACTF.Identity, bias=combc[:, :])
                omTg = omph.tile([128, BR, 27], BF16, tag="omT")
                pst = ompsum.tile([128, BR, 28], BF16, tag="pst")
                for j in range(BR):
                    nc.tensor.transpose(pst[:, j, 0:27], om_sb[:, j, :],
                                        identb[0:27, 0:27])
                nc.scalar.copy(omTg[:, :, :], pst[:, :, 0:27])
                return omTg

            def emit_prep(omTg):
                mbig = mtmp.tile([128, 8, KK, BR], F32, tag="mbig")
                dyT, dxT, sgT, ey, ly, ay, f0, s = [mbig[:, i] for i in range(8)]
                wm_t = mtmp.tile([128, KK, BR], F32, tag="wm")
                wm = wm_t[:, :, :]
                w0_t = mtmp.tile([128, KK, BR], F32, tag="w0")
                w0 = w0_t[:, :, :]
                wp_t = mtmp.tile([128, KK, BR], F32, tag="wp")
                wp = wp_t[:, :, :]
                wys = mtmp.tile([128, KK, 3, BR], BF16, tag="wys")
                wxs = mtmp.tile([128, KK, 3, BR], BF16, tag="wxs")
                masks = mout.tile([128, KK, 3, 3, BR], BF16, tag="masks")
                for dst, lo in [(dyT, 0), (dxT, 9), (sgT, 18)]:
                    nc.vector.tensor_copy(dst[:, :, :],
                                          omTg[:, :, lo : lo + 9].rearrange("p y c -> p c y"))
                nc.scalar.activation(sgT, sgT, ACTF.Sigmoid)
                for dT, wtile, fold_sig in [(dyT, wys, True), (dxT, wxs, False)]:
                    nc.vector.tensor_scalar(out=dT[:, :, :], in0=dT[:, :, :], scalar1=-CLAMP,
                                            scalar2=CLAMP, op0=ALU.max, op1=ALU.min)
                    nc.vector.tensor_scalar(out=ey, in0=dT[:, :, :], scalar1=0.0,
                                            scalar2=None, op0=ALU.is_lt)
                    nc.vector.tensor_tensor(out=ly, in0=dT[:, :, :], in1=ey, op=ALU.add)
                    nc.scalar.activation(ay, ly, ACTF.Identity, bias=1.0, scale=-1.0)
                    nc.scalar.activation(f0, ey, ACTF.Identity, bias=1.0, scale=-1.0)
                    nc.vector.tensor_tensor(out=wm, in0=ey, in1=ay, op=ALU.mult)
                    nc.vector.tensor_tensor(out=wp, in0=f0, in1=ly, op=ALU.mult)
                    nc.vector.tensor_tensor(out=s, in0=wm, in1=wp, op=ALU.add)
                    nc.scalar.activation(w0, s, ACTF.Identity, bias=1.0, scale=-1.0)
                    for k, wk in enumerate([wm, w0, wp]):
                        if fold_sig:
                            nc.vector.tensor_tensor(out=wtile[:, :, k, :], in0=wk[:, :, :],
                                                    in1=sgT, op=ALU.mult)
                        else:
                            nc.vector.tensor_copy(wtile[:, :, k, :], wk[:, :, :])
                nc.vector.tensor_tensor(
                    out=masks[:, :, :, :, :],
                    in0=wys[:, :, :, None, :].broadcast_to([128, KK, 3, 3, BR]),
                    in1=wxs[:, :, None, :, :].broadcast_to([128, KK, 3, 3, BR]),
                    op=ALU.mult)
                return masks

            def emit_apply(g, masks):
                y0 = BR * g
                vt = vpool.tile([128, 5, 128, BR], BF16, tag="vt")
                val_blk = vblk.tile([128, 5, BR, W], BF16, tag="vb")
                nc.gpsimd.memset(vt[:, 4, 64:128, :], 0.0)
                for p in range(KK):
                    ky, kx = p // 3, p % 3
                    r0 = y0 + ky + 1
                    t, pp = p // 2, p % 2
                    prodbuf = papool.tile([128, KK, CIN, BR], BF16, tag="pb")
                    for sy in range(3):
                        # free dims (sx, c, y): y packed last; mask c-broadcast
                        # is a 0-stride middle dim -> DVE 2x mode
                        nc.vector.tensor_tensor(
                            out=prodbuf[:, 3 * sy : 3 * sy + 3, :, :],
                            in0=featT5[:, kx : kx + 3, :, r0 + sy : r0 + sy + BR],
                            in1=masks[:, p, sy, :, None, :]
                            .broadcast_to([128, 3, CIN, BR]),
                            op=ALU.mult)
                    acc = vt[:, t, 64 * pp : 64 * pp + 64, :]
                    s1 = papool.tile([128, 4, CIN, BR], BF16, tag="s1")
                    nc.vector.tensor_tensor(out=s1[:, :, :, :], in0=prodbuf[:, 0:4],
                                            in1=prodbuf[:, 4:8], op=ALU.add)
                    nc.vector.tensor_tensor(out=s1[:, 0:2], in0=s1[:, 0:2],
                                            in1=s1[:, 2:4], op=ALU.add)
                    nc.vector.tensor_tensor(out=acc, in0=s1[:, 0], in1=s1[:, 1],
                                            op=ALU.add)
                    nc.vector.tensor_tensor(out=acc, in0=acc, in1=prodbuf[:, 8],
                                            op=ALU.add)
                    if p % 2 == 1 or p == KK - 1:
                        t_done = p // 2
                        for hh in range(2):
                            pst2 = btps.tile([128, 8 * 128], BF16, tag="bt")
                            for j8 in range(8):
                                j = 8 * hh + j8
                                nc.tensor.transpose(pst2[:, 128 * j8 : 128 * (j8 + 1)],
                                                    vt[:, t_done, :, j], identb[:, :])
                            nc.scalar.copy(val_blk[:, t_done, 8 * hh : 8 * hh + 8, :],
                                           pst2[:, :])
                oc = och.tile([COUT, BR, W], F32, tag="oc")
                for c2 in range(BR // 4):
                    ps2 = mmps.tile([COUT, 512], F32, tag="mm")
                    for t in range(5):
                        nc.tensor.matmul(ps2[:, :], w2b[:, t, :],
                                         val_blk[:, t, 4 * c2 : 4 * c2 + 4, :],
                                         start=(t == 0), stop=(t == 4))
                    nc.scalar.activation(oc[:, 4 * c2 : 4 * c2 + 4, :], ps2[:, :],
                                         ACTF.Identity, bias=bias2c[:, :])
                nc.sync.dma_start(out=d_out[:, y0 : y0 + BR, :], in_=oc[:, :, :])

            # software-pipelined emission: om/feat for block g+1 land on the PE
            # while the DVE chews block g; mask-prep g+1 follows apply g on DVE
            emit_ft([0])
            omT0 = emit_om(0)
            masks_g = emit_prep(omT0)
            emit_ft([1, 2])
            # deferred w2b prep (needed only by the first c2-matmuls @ ~150us);
            # fvec = c1_w @ fea -> [64, 1], computed into col 0 of an "mm" tile
            ps_fv = mmps.tile([COUT, 512], F32, tag="mm")
            for k in range(2):
                nc.tensor.matmul(ps_fv[:, 0:1], c1wb[:, k, :], feab[:, k : k + 1],
                                 start=(k == 0), stop=(k == 1))
            nc.scalar.copy(fvec[0:COUT, :], ps_fv[:, 0:1])
            nc.sync.dma_start(out=fvec[COUT:128, :], in_=fvec[0:COUT, :])
            # w2b = (w2cp * fvec[c]) cast to bf16
            nc.vector.tensor_scalar(out=w2b.rearrange("p a b -> p (a b)")[:, :],
                                    in0=w2cp[:, :], scalar1=fvec[:, :], scalar2=None,
                                    op0=ALU.mult)
            for g in range(NOUT // BR):
                if g < 3:
                    omTn = emit_om(g + 1)
                    emit_ft([2 * g + 3, 2 * g + 4])
                cur = masks_g
                emit_apply(g, cur)
                if g < 3:
                    masks_g = emit_prep(omTn)

    nc.compile()
    return nc


def _host_prep(inputs):
    """Build the 8 per-core input maps (numpy marshalling only)."""
    feat = np.ascontiguousarray(inputs["input_feat"], dtype=np.float32)
    inter = np.ascontiguousarray(inputs["inter"], dtype=np.float32)
    fea = np.asarray(inputs["fea"], dtype=np.float32)[:, :, 0, 0]  # [B, 256]
    weight = np.asarray(inputs["weight"], dtype=np.float32)
    bias = np.asarray(inputs["bias"], dtype=np.float32)
    com_w = np.asarray(inputs["com_w"], dtype=np.float32)
    com_b = np.asarray(inputs["com_b"], dtype=np.float32)
    c1_w = np.asarray(inputs["c1_w"], dtype=np.float32)
    c2_w = np.asarray(inputs["c2_w"], dtype=np.float32)

    # fold c2 into the static weight:  weight2[o2, c, p] (parameter prep)
    w_r = weight.reshape(COUT, CIN, KK)
    weight2 = np.einsum("ao,ocp->acp", c2_w, w_r)  # [64, 64, 9]
    w2 = np.zeros((128, 5, 64), np.float32)  # [(c, p-pair), ktile, o2]
    for p in range(KK):
        t, pp = p // 2, p % 2
        w2[64 * pp : 64 * pp + 64, t, :] = weight2[:, :, p].T  # [c, o2]
    bias2 = (c2_w @ bias).reshape(COUT, 1)

    # com_w reordered: channels [dy x9, dx x9, sig x9]; layout [cin, tap, 27]
    perm = list(range(0, 18, 2)) + list(range(1, 18, 2)) + list(range(18, 27))
    comw = np.ascontiguousarray(
        com_w[perm].reshape(27, CIN, KK).transpose(1, 2, 0))  # [CIN, KK, 27]
    comb = com_b[perm].reshape(27, 1).astype(np.float32)

    c1w = np.ascontiguousarray(c1_w.T.reshape(2, 128, COUT).transpose(1, 0, 2))
    ident = np.eye(128, dtype=np.float32)

    in_maps = []
    for i in range(8):
        b, h = i // 2, i % 2
        r0 = NOUT * h
        fpad = np.zeros((CIN, NR, 132), np.float32)
        glo, ghi = r0 - 3, r0 - 3 + NR
        slo, shi = max(0, glo), min(H, ghi)
        fpad[:, slo - glo : shi - glo, 2 : 2 + W] = feat[b, :, slo:shi, :]
        ipad = np.zeros((CIN, NRI, 130), np.float32)
        glo, ghi = r0 - 1, r0 - 1 + NRI
        slo, shi = max(0, glo), min(H, ghi)
        ipad[:, slo - glo : shi - glo, 1 : 1 + W] = inter[b, :, slo:shi, :]
        feac = np.ascontiguousarray(fea[b].reshape(2, 128).T)
        in_maps.append(dict(feat=fpad, inter=ipad, w2=w2, comw=comw, comb=comb,
                            c1w=c1w, fea=feac, bias2=bias2, ident=ident))
    return in_maps


def kernel(**inputs) -> np.ndarray:
    if "nc" not in _CACHED:
        _CACHED["nc"] = _build_nc()
    nc = _CACHED["nc"]
    in_maps = _host_prep(inputs)
    res = run_bass_kernel_spmd(nc, in_maps, core_ids=list(range(8)),
                               **_CACHED.get("run_kwargs", {}))
    _CACHED["last_result"] = res
    out = np.zeros((B, COUT, H, W), np.float32)
    for i in range(8):
        b, h = i // 2, i % 2
        out[b, :, NOUT * h : NOUT * (h + 1), :] = res.results[i]["out"]
    return out



# revision 24
# speedup vs baseline: 1.6191x; 1.0027x over previous
"""Trainium2 Bass kernel for the DCN (modulated deformable conv) layer.

Self-contained: hardcodes all shapes. Shards data-parallel over (batch b x
row-half h) onto 8 NeuronCores; each core computes a [64, 64, 128] slab of
the [4, 64, 128, 128] output.

Per-core algorithm (all position indices x live on SBUF partitions):
  1. om-conv (3x3, PE, bf16): offsets dy,dx + mask logits from `inter`.
  2. PE-transpose om -> omT[x, y, ch]; build bilinear/sigmoid mask planes
     m[p, sy, sx, y][x] on DVE (positions-in-partitions layout).
  3. PE-transpose feat -> featT5[x, dx_shift, row, c] (5 column shifts via
     DMA partition-shifted copies).
  4. Apply: per (tap, 4-row block): 9 contiguous tensor_tensor products
     (mask broadcast along c via a 0-stride free dim) + an 8-op add chain
     on DVE -- 2448 ops total, all operands contiguous [128, 4, 64] bf16.
  5. PE-transpose valT back to val[(c,p)-partitions, (y,x)] and contract with
     the per-sample dynamic weights W2' = c2_w @ (weight * fvec) on PE.
"""

import numpy as np
from contextlib import ExitStack

import concourse.bass as bass
import concourse.bacc as bacc
import concourse.tile as tile
from concourse import mybir
from concourse.bass_utils import run_bass_kernel_spmd

F32 = mybir.dt.float32
BF16 = mybir.dt.bfloat16
ALU = mybir.AluOpType
ACTF = mybir.ActivationFunctionType

B, CIN, COUT, H, W, K = 4, 64, 64, 128, 128, 3
KK = K * K
NOUT = 64          # out rows per core
NR = 69            # feat rows resident per core:  y + ky-1 + sy in [-3, 66)
NRI = 66           # inter rows resident (conv halo 1)
NCH = 16           # position chunks (of 512) per core
CLAMP = 0.999999

_CACHED = {}


def _build_nc():
    nc = bacc.Bacc("TRN2", target_bir_lowering=False)

    # ---- DRAM I/O (per-core views; same program on all 8 cores) ----
    d_feat = nc.dram_tensor("feat", [CIN, NR, 132], F32, kind="ExternalInput")
    d_inter = nc.dram_tensor("inter", [CIN, NRI, 130], F32, kind="ExternalInput")
    d_w2 = nc.dram_tensor("w2", [128, 5, 64], F32, kind="ExternalInput")
    d_comw = nc.dram_tensor("comw", [CIN, KK, 27], F32, kind="ExternalInput")
    d_comb = nc.dram_tensor("comb", [27, 1], F32, kind="ExternalInput")
    d_c1w = nc.dram_tensor("c1w", [128, 2, COUT], F32, kind="ExternalInput")
    d_fea = nc.dram_tensor("fea", [128, 2], F32, kind="ExternalInput")
    d_bias2 = nc.dram_tensor("bias2", [COUT, 1], F32, kind="ExternalInput")
    d_ident = nc.dram_tensor("ident", [128, 128], F32, kind="ExternalInput")
    d_out = nc.dram_tensor("out", [COUT, NOUT, W], F32, kind="ExternalOutput")

    with ExitStack() as ctx:
        tc = ctx.enter_context(tile.TileContext(nc))

        # ---------------- persistent small pool ----------------
        pers = ctx.enter_context(tc.tile_pool(name="pers", bufs=1))
        identb = pers.tile([128, 128], BF16)
        w2cp = pers.tile([128, 5 * 64], F32)
        w2b = pers.tile([128, 5, 64], BF16)
        comwb = pers.tile([CIN, KK, 27], BF16)
        combc = pers.tile([27, 1], F32)
        c1wb = pers.tile([128, 2, COUT], BF16)
        feab = pers.tile([128, 2], BF16)
        bias2c = pers.tile([COUT, 1], F32)
        fvec = pers.tile([128, 1], F32)

        identf = pers.tile([128, 128], F32)
        nc.sync.dma_start(out=identf, in_=d_ident[:, :])
        nc.vector.tensor_copy(identb[:, :], identf[:, :])
        nc.sync.dma_start(out=w2cp, in_=d_w2.rearrange("p a b -> p (a b)")[:, :])
        nc.sync.dma_start(out=combc, in_=d_comb[:, :])
        nc.sync.dma_start(out=bias2c, in_=d_bias2[:, :])

        # ---------------- phase A setup: featT5 written directly in 5 shifted
        # planes by transposing padded cols di:di+128 (no SBUF shift DMAs) ----
        featT5 = pers.tile([128, 5, CIN, NR], BF16)
        interb = pers.tile([CIN, NRI, 130], BF16)
        featb = pers.tile([CIN, NR, 132], BF16)
        nc.gpsimd.dma_start(out=interb, in_=d_inter[:, :, :])
        nc.gpsimd.dma_start(out=featb, in_=d_feat[:, :, :])
        nc.gpsimd.dma_start(out=comwb, in_=d_comw[:, :, :])  # cast f32->bf16
        nc.gpsimd.dma_start(out=c1wb, in_=d_c1w[:, :, :])
        nc.gpsimd.dma_start(out=feab, in_=d_fea[:, :])

        # ------- pipelined per-16-row blocks: om conv -> masks -> apply -------
        BR = 16
        with (
            tc.tile_pool(name="omph", bufs=2) as omph,
            tc.tile_pool(name="mtmp", bufs=1) as mtmp,
            tc.tile_pool(name="mout", bufs=2) as mout,
            tc.tile_pool(name="vpool", bufs=2) as vpool,
            tc.tile_pool(name="papool", bufs=1) as papool,
            tc.tile_pool(name="vblk", bufs=1) as vblk,
            tc.tile_pool(name="och", bufs=1) as och,
            tc.tile_pool(name="ompsum", bufs=1, space="PSUM") as ompsum,
            tc.tile_pool(name="ftps", bufs=2, space="PSUM") as ftps,
            tc.tile_pool(name="btps", bufs=2, space="PSUM") as btps,
            tc.tile_pool(name="mmps", bufs=2, space="PSUM") as mmps,
        ):
            def emit_ft(groups):
                # transpose feat rows into all 5 shifted planes: plane di takes
                # padded cols di:di+128 (host zero-pad covers the edges)
                for g9 in groups:
                    rows = range(8 * g9, min(8 * g9 + 8, NR))
                    for di in range(5):
                        pstf = ftps.tile([128, 8 * CIN], BF16, tag="ftp")
                        for j, r in enumerate(rows):
                            nc.tensor.transpose(pstf[:, CIN * j : CIN * (j + 1)],
                                                featb[:, r, di : di + 128],
                                                identb[0:CIN, 0:CIN])
                        nc.scalar.copy(
                            featT5[:, di, :, 8 * g9 : 8 * g9 + len(rows)],
                            pstf[:, : len(rows) * CIN]
                            .rearrange("p (j c) -> p c j", j=len(rows)))

            def emit_om(g):
                y0 = BR * g
                om_sb = omph.tile([27, BR, W], BF16, tag="omsb")
                for nch in range(BR // 4):
                    ps = ompsum.tile([27, 512], F32, tag="ps")
                    yy = y0 + 4 * nch
                    for d in range(KK):
                        dy, dx = d // 3, d % 3
                        rhs = interb[:, yy + dy : yy + dy + 4, dx : dx + W]
                        nc.tensor.matmul(ps[:, :], comwb[:, d, :], rhs,
                                         start=(d == 0), stop=(d == KK - 1))
                    nc.scalar.activation(
                        om_sb[:, 4 * nch : 4 * nch + 4, :].rearrange("p a b -> p (a b)"),
                        ps[:, :], ACTF.Identity, bias=combc[:, :])
                omTg = omph.tile([128, BR, 27], BF16, tag="omT")
                pst = ompsum.tile([128, BR, 28], BF16, tag="pst")
                for j in range(BR):
                    nc.tensor.transpose(pst[:, j, 0:27], om_sb[:, j, :],
                                        identb[0:27, 0:27])
                nc.scalar.copy(omTg[:, :, :], pst[:, :, 0:27])
                return omTg

            def emit_prep(omTg):
                mbig = mtmp.tile([128, 8, KK, BR], F32, tag="mbig")
                dyT, dxT, sgT, ey, ly, ay, f0, s = [mbig[:, i] for i in range(8)]
                wm_t = mtmp.tile([128, KK, BR], F32, tag="wm")
                wm = wm_t[:, :, :]
                w0_t = mtmp.tile([128, KK, BR], F32, tag="w0")
                w0 = w0_t[:, :, :]
                wp_t = mtmp.tile([128, KK, BR], F32, tag="wp")
                wp = wp_t[:, :, :]
                wys = mtmp.tile([128, KK, 3, BR], BF16, tag="wys")
                wxs = mtmp.tile([128, KK, 3, BR], BF16, tag="wxs")
                masks = mout.tile([128, KK, 3, 3, BR], BF16, tag="masks")
                for dst, lo in [(dyT, 0), (dxT, 9), (sgT, 18)]:
                    nc.vector.tensor_copy(dst[:, :, :],
                                          omTg[:, :, lo : lo + 9].rearrange("p y c -> p c y"))
                nc.scalar.activation(sgT, sgT, ACTF.Sigmoid)
                for dT, wtile, fold_sig in [(dyT, wys, True), (dxT, wxs, False)]:
                    nc.vector.tensor_scalar(out=dT[:, :, :], in0=dT[:, :, :], scalar1=-CLAMP,
                                            scalar2=CLAMP, op0=ALU.max, op1=ALU.min)
                    nc.vector.tensor_scalar(out=ey, in0=dT[:, :, :], scalar1=0.0,
                                            scalar2=None, op0=ALU.is_lt)
                    nc.vector.tensor_tensor(out=ly, in0=dT[:, :, :], in1=ey, op=ALU.add)
                    nc.scalar.activation(ay, ly, ACTF.Identity, bias=1.0, scale=-1.0)
                    nc.scalar.activation(f0, ey, ACTF.Identity, bias=1.0, scale=-1.0)
                    nc.vector.tensor_tensor(out=wm, in0=ey, in1=ay, op=ALU.mult)
                    nc.vector.tensor_tensor(out=wp, in0=f0, in1=ly, op=ALU.mult)
                    nc.vector.tensor_tensor(out=s, in0=wm, in1=wp, op=ALU.add)
                    nc.scalar.activation(w0, s, ACTF.Identity, bias=1.0, scale=-1.0)
                    for k, wk in enumerate([wm, w0, wp]):
                        if fold_sig:
                            nc.vector.tensor_tensor(out=wtile[:, :, k, :], in0=wk[:, :, :],
                                                    in1=sgT, op=ALU.mult)
                        else:
                            nc.vector.tensor_copy(wtile[:, :, k, :], wk[:, :, :])
                nc.vector.tensor_tensor(
                    out=masks[:, :, :, :, :],
                    in0=wys[:, :, :, None, :].broadcast_to([128, KK, 3, 3, BR]),
                    in1=wxs[:, :, None, :, :].broadcast_to([128, KK, 3, 3, BR]),
                    op=ALU.mult)
                return masks

            def emit_apply(g, masks):
                y0 = BR * g
                vt = vpool.tile([128, 5, 128, BR], BF16, tag="vt")
                val_blk = vblk.tile([128, 5, BR, W], BF16, tag="vb")
                oc = och.tile([COUT, BR, W], F32, tag="oc")
                nc.gpsimd.memset(vt[:, 4, 64:128, :], 0.0)
                for p in range(KK):
                    ky, kx = p // 3, p % 3
                    r0 = y0 + ky + 1
                    t, pp = p // 2, p % 2
                    prodbuf = papool.tile([128, KK, CIN, BR], BF16, tag="pb")
                    for sy in range(3):
                        # free dims (sx, c, y): y packed last; mask c-broadcast
                        # is a 0-stride middle dim -> DVE 2x mode
                        nc.vector.tensor_tensor(
                            out=prodbuf[:, 3 * sy : 3 * sy + 3, :, :],
                            in0=featT5[:, kx : kx + 3, :, r0 + sy : r0 + sy + BR],
                            in1=masks[:, p, sy, :, None, :]
                            .broadcast_to([128, 3, CIN, BR]),
                            op=ALU.mult)
                    acc = vt[:, t, 64 * pp : 64 * pp + 64, :]
                    s1 = papool.tile([128, 4, CIN, BR], BF16, tag="s1")
                    nc.vector.tensor_tensor(out=s1[:, :, :, :], in0=prodbuf[:, 0:4],
                                            in1=prodbuf[:, 4:8], op=ALU.add)
                    nc.vector.tensor_tensor(out=s1[:, 0:2], in0=s1[:, 0:2],
                                            in1=s1[:, 2:4], op=ALU.add)
                    nc.vector.tensor_tensor(out=acc, in0=s1[:, 0], in1=s1[:, 1],
                                            op=ALU.add)
                    nc.vector.tensor_tensor(out=acc, in0=acc, in1=prodbuf[:, 8],
                                            op=ALU.add)
                    if p % 2 == 1 or p == KK - 1:
                        t_done = p // 2
                        for hh in range(2):
                            pst2 = btps.tile([128, 8 * 128], BF16, tag="bt")
                            for j8 in range(8):
                                j = 8 * hh + j8
                                nc.tensor.transpose(pst2[:, 128 * j8 : 128 * (j8 + 1)],
                                                    vt[:, t_done, :, j], identb[:, :])
                            nc.scalar.copy(val_blk[:, t_done, 8 * hh : 8 * hh + 8, :],
                                           pst2[:, :])
                            if p == KK - 1:
                                # rows of this half are complete across all t:
                                # contract + ship them while the other half runs
                                for c2 in range(2 * hh, 2 * hh + 2):
                                    ps2 = mmps.tile([COUT, 512], F32, tag="mm")
                                    for t in range(5):
                                        nc.tensor.matmul(
                                            ps2[:, :], w2b[:, t, :],
                                            val_blk[:, t, 4 * c2 : 4 * c2 + 4, :],
                                            start=(t == 0), stop=(t == 4))
                                    nc.scalar.activation(
                                        oc[:, 4 * c2 : 4 * c2 + 4, :], ps2[:, :],
                                        ACTF.Identity, bias=bias2c[:, :])
                                nc.sync.dma_start(
                                    out=d_out[:, y0 + 8 * hh : y0 + 8 * hh + 8, :],
                                    in_=oc[:, 8 * hh : 8 * hh + 8, :])

            # software-pipelined emission: om/feat for block g+1 land on the PE
            # while the DVE chews block g; mask-prep g+1 follows apply g on DVE
            omT0 = emit_om(0)
            masks_g = emit_prep(omT0)
            emit_ft([0, 1, 2])
            # deferred w2b prep (needed only by the first c2-matmuls @ ~150us);
            # fvec = c1_w @ fea -> [64, 1], computed into col 0 of an "mm" tile
            ps_fv = mmps.tile([COUT, 512], F32, tag="mm")
            for k in range(2):
                nc.tensor.matmul(ps_fv[:, 0:1], c1wb[:, k, :], feab[:, k : k + 1],
                                 start=(k == 0), stop=(k == 1))
            nc.scalar.copy(fvec[0:COUT, :], ps_fv[:, 0:1])
            nc.sync.dma_start(out=fvec[COUT:128, :], in_=fvec[0:COUT, :])
            # w2b = (w2cp * fvec[c]) cast to bf16
            nc.vector.tensor_scalar(out=w2b.rearrange("p a b -> p (a b)")[:, :],
                                    in0=w2cp[:, :], scalar1=fvec[:, :], scalar2=None,
                                    op0=ALU.mult)
            for g in range(NOUT // BR):
                if g < 3:
                    omTn = emit_om(g + 1)
                    emit_ft([2 * g + 3, 2 * g + 4])
                cur = masks_g
                emit_apply(g, cur)
                if g < 3:
                    masks_g = emit_prep(omTn)

    nc.compile()
    return nc


def _host_prep(inputs):
    """Build the 8 per-core input maps (numpy marshalling only)."""
    feat = np.ascontiguousarray(inputs["input_feat"], dtype=np.float32)
    inter = np.ascontiguousarray(inputs["inter"], dtype=np.float32)
    fea = np.asarray(inputs["fea"], dtype=np.float32)[:, :, 0, 0]  # [B, 256]
    weight = np.asarray(inputs["weight"], dtype=np.float32)
    bias = np.asarray(inputs["bias"], dtype=np.float32)
    com_w = np.asarray(inputs["com_w"], dtype=np.float32)
    com_b = np.asarray(inputs["com_b"], dtype=np.float32)
    c1_w = np.asarray(inputs["c1_w"], dtype=np.float32)
    c2_w = np.asarray(inputs["c2_w"], dtype=np.float32)

    # fold c2 into the static weight:  weight2[o2, c, p] (parameter prep)
    w_r = weight.reshape(COUT, CIN, KK)
    weight2 = np.einsum("ao,ocp->acp", c2_w, w_r)  # [64, 64, 9]
    w2 = np.zeros((128, 5, 64), np.float32)  # [(c, p-pair), ktile, o2]
    for p in range(KK):
        t, pp = p // 2, p % 2
        w2[64 * pp : 64 * pp + 64, t, :] = weight2[:, :, p].T  # [c, o2]
    bias2 = (c2_w @ bias).reshape(COUT, 1)

    # com_w reordered: channels [dy x9, dx x9, sig x9]; layout [cin, tap, 27]
    perm = list(range(0, 18, 2)) + list(range(1, 18, 2)) + list(range(18, 27))
    comw = np.ascontiguousarray(
        com_w[perm].reshape(27, CIN, KK).transpose(1, 2, 0))  # [CIN, KK, 27]
    comb = com_b[perm].reshape(27, 1).astype(np.float32)

    c1w = np.ascontiguousarray(c1_w.T.reshape(2, 128, COUT).transpose(1, 0, 2))
    ident = np.eye(128, dtype=np.float32)

    in_maps = []
    for i in range(8):
        b, h = i // 2, i % 2
        r0 = NOUT * h
        fpad = np.zeros((CIN, NR, 132), np.float32)
        glo, ghi = r0 - 3, r0 - 3 + NR
        slo, shi = max(0, glo), min(H, ghi)
        fpad[:, slo - glo : shi - glo, 2 : 2 + W] = feat[b, :, slo:shi, :]
        ipad = np.zeros((CIN, NRI, 130), np.float32)
        glo, ghi = r0 - 1, r0 - 1 + NRI
        slo, shi = max(0, glo), min(H, ghi)
        ipad[:, slo - glo : shi - glo, 1 : 1 + W] = inter[b, :, slo:shi, :]
        feac = np.ascontiguousarray(fea[b].reshape(2, 128).T)
        in_maps.append(dict(feat=fpad, inter=ipad, w2=w2, comw=comw, comb=comb,
                            c1w=c1w, fea=feac, bias2=bias2, ident=ident))
    return in_maps


def kernel(**inputs) -> np.ndarray:
    if "nc" not in _CACHED:
        _CACHED["nc"] = _build_nc()
    nc = _CACHED["nc"]
    in_maps = _host_prep(inputs)
    res = run_bass_kernel_spmd(nc, in_maps, core_ids=list(range(8)),
                               **_CACHED.get("run_kwargs", {}))
    _CACHED["last_result"] = res
    out = np.zeros((B, COUT, H, W), np.float32)
    for i in range(8):
        b, h = i // 2, i % 2
        out[b, :, NOUT * h : NOUT * (h + 1), :] = res.results[i]["out"]
    return out

